# revision 3
# baseline (speedup 1.0000x reference)
"""LocalAttention Trainium2 kernel — 8-core data-parallel over batch.

Reference math:
  ld   = relu(LD @ Wd + bd);  gate-applied neighbors an = AN * ld
  q    = AQ @ Wq + bq
  k    = an @ Wk + bk ; v = an @ Wv + bv     (per neighbor)
  e    = m * (qh * kh) * D^-0.5              (per-channel energy)
  attn = softmax(e, axis=n)
  ctx  = sum_n attn * (m*vh) + q

Mask folding (m in {0,1} guaranteed by the input spec):
  m*relu(LD@Wd + bd) == relu((m*LD)@Wd + bd*m)   and with anm = AN*ld*m:
  m*k = anm@Wk + m*bk,  m*v = anm@Wv + m*bv.
The bd*m / m*bk / m*bv rank-1 terms are added via K=1 matmuls with the
transposed mask row — only emitted when the bias is nonzero (it is zero
for this workload, so the fast path is exact and bias-free). bq is always
added exactly in fp32.
"""

import sys

sys.path.insert(0, "/opt/trn_rl_repo")

import numpy as np
import ml_dtypes

import concourse.bass as bass
import concourse.mybir as mybir
import concourse.tile as tile
from concourse import bacc
from concourse.masks import make_identity
from concourse.bass_utils import run_bass_kernel_spmd

BF16 = mybir.dt.bfloat16
F32 = mybir.dt.float32

NC = 8          # cores
BS = 32         # full batch
NB = BS // NC   # batches per core
Q = 128
N = 32          # neighbors
H = 8           # heads
D = 32          # head dim
F = 256         # feature width (= H*D)
DD = 64         # distance feature width
SCALE = float(D ** -0.5)

_cache = {}


def _mk_ap(base_ap, extra_offset, dims):
    """AP on base_ap.tensor at base offset + extra, dims = [(step, count), ...]
    (first entry is the partition dim)."""
    return bass.AP(
        tensor=base_ap.tensor,
        offset=base_ap.offset + extra_offset,
        ap=[[s, c] for (s, c) in dims],
    )


def _build(with_bd: bool, with_bkbv: bool):
    nc = bacc.Bacc("TRN2", target_bir_lowering=False, debug=False, num_devices=NC)

    an_t = nc.dram_tensor("an_t", [NB, 2, 128, N, Q], BF16, kind="ExternalInput")
    ld_p = nc.dram_tensor("ld_p", [NB, Q, DD, N], BF16, kind="ExternalInput")
    msk = nc.dram_tensor("msk", [NB, Q, N], BF16, kind="ExternalInput")
    aq = nc.dram_tensor("aq", [NB, Q, F], F32, kind="ExternalInput")
    wd = nc.dram_tensor("wd", [DD, F], BF16, kind="ExternalInput")
    wkwv = nc.dram_tensor("wkwv", [F, 512], BF16, kind="ExternalInput")
    wq = nc.dram_tensor("wq", [F, F], F32, kind="ExternalInput")
    bq_d = nc.dram_tensor("bq_d", [F], F32, kind="ExternalInput")
    bd_bf_d = nc.dram_tensor("bd_bf_d", [F], BF16, kind="ExternalInput")
    bkbv_d = nc.dram_tensor("bkbv_d", [512], BF16, kind="ExternalInput")

    attn_o = nc.dram_tensor("attn_o", [NB, H, Q, N * D], BF16, kind="ExternalOutput")
    ctx_o = nc.dram_tensor("ctx_o", [NB, Q, F], F32, kind="ExternalOutput")

    AT = mybir.ActivationFunctionType

    with tile.TileContext(nc) as tc, \
            tc.tile_pool(name="const", bufs=1) as const, \
            tc.tile_pool(name="ldpool", bufs=2) as ldpool, \
            tc.tile_pool(name="dtpool", bufs=2) as dtpool, \
            tc.tile_pool(name="gpool", bufs=1) as gpool, \
            tc.tile_pool(name="antpool", bufs=2) as antpool, \
            tc.tile_pool(name="qpool", bufs=2) as qpool, \
            tc.tile_pool(name="epool", bufs=1) as epool, \
            tc.tile_pool(name="vpool", bufs=1) as vpool, \
            tc.tile_pool(name="ppool", bufs=1) as ppool, \
            tc.tile_pool(name="tmppool", bufs=1) as tmppool, \
            tc.tile_pool(name="attnpool", bufs=2) as attnpool, \
            tc.tile_pool(name="smallpool", bufs=2) as smallpool, \
            tc.tile_pool(name="tpsum", bufs=2, space="PSUM") as tpsum, \
            tc.tile_pool(name="mpsum", bufs=2, space="PSUM") as mpsum, \
            tc.tile_pool(name="kvpsum", bufs=2, space="PSUM") as kvpsum:

        # ---------------- constants ----------------
        wd_sb = const.tile([DD, F], BF16)
        nc.sync.dma_start(out=wd_sb[:, :], in_=wd[:, :])
        wkwv_sb = const.tile([128, 2, 512], BF16)
        nc.sync.dma_start(
            out=wkwv_sb[:, :, :],
            in_=wkwv[:, :].rearrange("(c f) n -> f c n", c=2),
        )
        wq_sb = const.tile([128, 2, F], F32)
        nc.sync.dma_start(
            out=wq_sb[:, :, :],
            in_=wq[:, :].rearrange("(c f) n -> f c n", c=2),
        )
        bq_sb = const.tile([128, F], F32)
        nc.sync.dma_start(
            out=bq_sb[:, :],
            in_=_mk_ap(bq_d[:], 0, [(0, 128), (1, F)]),
        )
        idn_bf = const.tile([128, 128], BF16)
        make_identity(nc, idn_bf[:, :])
        idn_f32 = const.tile([128, 128], F32)
        make_identity(nc, idn_f32[:, :])
        if with_bd:
            bd_bf_sb = const.tile([1, F], BF16)
            nc.sync.dma_start(out=bd_bf_sb[:, :], in_=bd_bf_d[None, :])
        if with_bkbv:
            bkbv_sb = const.tile([1, 512], BF16)
            nc.sync.dma_start(out=bkbv_sb[:, :], in_=bkbv_d[None, :])

        for b in range(NB):
            # ---------------- distance gate path ----------------
            ld_sb = ldpool.tile([Q, DD, N], BF16, tag="ld")
            nc.sync.dma_start(out=ld_sb[:, :, :], in_=ld_p[b])
            m_sb = smallpool.tile([Q, N], BF16, tag="m")
            nc.sync.dma_start(out=m_sb[:, :], in_=msk[b])
            if with_bd or with_bkbv:
                # mask in (n, q) order on one partition, for rank-1 bias terms
                mrow_sb = smallpool.tile([1, N, Q], BF16, tag="mrow")
                nc.sync.dma_start(
                    out=mrow_sb[:, :, :],
                    in_=_mk_ap(msk[:], b * Q * N, [(0, 1), (1, N), (N, Q)]),
                )

            # LDm = LD * mask (mask broadcast over dd; n innermost)
            ldm_sb = ldpool.tile([Q, DD, N], BF16, tag="ldm")
            nc.vector.tensor_mul(
                ldm_sb[:, :, :],
                ld_sb[:, :, :],
                _mk_ap(m_sb[:], 0, [(N, Q), (0, DD), (1, N)]),
            )

            # transpose per neighbor: [q, dd] -> [dd, q]; 4 per PSUM bank
            dT_sb = dtpool.tile([DD, N * Q], BF16, tag="dT")
            for g in range(N // 4):
                tp = tpsum.tile([DD, 4, Q], BF16, tag="tp")
                for j in range(4):
                    nc.tensor.transpose(
                        tp[:, j, :], ldm_sb[:, :, g * 4 + j], idn_bf[:, :]
                    )
                nc.scalar.copy(
                    dT_sb[:, g * 4 * Q : (g + 1) * 4 * Q], tp[:, :, :]
                )

            # gate = relu(Wd_c.T @ dT [+ bd x m]) -> [fout 2x128, (n,q)]
            g_sb = gpool.tile([128, 2, N * Q], BF16, tag="g")
            for c in range(2):
                for g in range(8):
                    mm = mpsum.tile([128, 512], F32, tag="mp")
                    nc.tensor.matmul(
                        mm[:, :],
                        wd_sb[:, c * 128 : (c + 1) * 128],
                        dT_sb[:, g * 512 : (g + 1) * 512],
                        start=True, stop=not with_bd,
                    )
                    if with_bd:
                        nc.tensor.matmul(
                            mm[:, :],
                            bd_bf_sb[:, c * 128 : (c + 1) * 128],
                            _mk_ap(mrow_sb[:], g * 512, [(0, 1), (1, 512)]),
                            start=False, stop=True,
                        )
                    nc.scalar.activation(
                        g_sb[:, c, g * 512 : (g + 1) * 512], mm[:, :], AT.Relu
                    )

            # ---------------- query path (fp32, exact) ----------------
            aq_sb = qpool.tile([Q, F], F32, tag="aq")
            nc.sync.dma_start(out=aq_sb[:, :], in_=aq[b])
            aqt_sb = qpool.tile([128, 2, 128], F32, tag="aqt")
            for c in range(2):
                tq = mpsum.tile([128, 512], F32, tag="mp")
                nc.tensor.transpose(
                    tq[:, 0:128], aq_sb[:, c * 128 : (c + 1) * 128], idn_f32[:, :]
                )
                nc.scalar.copy(aqt_sb[:, c, :], tq[:, 0:128])
            qp = mpsum.tile([128, 512], F32, tag="mp")
            for c in range(2):
                nc.tensor.matmul(
                    qp[:, 0:F], aqt_sb[:, c, :], wq_sb[:, c, :],
                    start=(c == 0), stop=(c == 1),
                )
            qf_sb = qpool.tile([Q, F], F32, tag="qf")
            nc.vector.tensor_add(qf_sb[:, :], qp[:, 0:F], bq_sb[:, :])

            # ---------------- gated neighbors ----------------
            ant_sb = antpool.tile([128, 2, N * Q], BF16, tag="ant")
            nc.sync.dma_start(
                out=ant_sb[:, :, :], in_=an_t[b].rearrange("c f n q -> f c (n q)")
            )
            for c in range(2):
                nc.vector.tensor_mul(
                    ant_sb[:, c, :], ant_sb[:, c, :], g_sb[:, c, :]
                )

            # ---------------- K|V projections, energy, softmax ----------
            e_sb = epool.tile([Q, H, N, D], BF16, tag="e")
            v_sb = vpool.tile([Q, H, N, D], BF16, tag="v")
            for g in range(N // 2):
                kv = kvpsum.tile([Q, 2, 512], F32, tag="kv")
                for j in range(2):
                    n = g * 2 + j
                    for c in range(2):
                        nc.tensor.matmul(
                            kv[:, j, :],
                            ant_sb[:, c, n * Q : (n + 1) * Q],
                            wkwv_sb[:, c, :],
                            start=(c == 0),
                            stop=(c == 1) and not with_bkbv,
                        )
                    if with_bkbv:
                        nc.tensor.matmul(
                            kv[:, j, :],
                            mrow_sb[:, n, :],
                            bkbv_sb[:, :],
                            start=False, stop=True,
                        )
                # e[:, :, 2g:2g+2, :] = k * q   (k = kv[:, :, 0:256])
                nc.vector.tensor_mul(
                    e_sb[:, :, g * 2 : g * 2 + 2, :],
                    _mk_ap(kv[:], 0, [(1024, Q), (D, H), (512, 2), (1, D)]),
                    _mk_ap(qf_sb[:], 0, [(F, Q), (D, H), (0, 2), (1, D)]),
                )
                # v evacuation (bf16)
                nc.scalar.copy(
                    v_sb[:, :, g * 2 : g * 2 + 2, :],
                    _mk_ap(kv[:], F, [(1024, Q), (D, H), (512, 2), (1, D)]),
                )

            # exp in place, scale folded into activation
            nc.scalar.activation(
                e_sb[:, :, :, :], e_sb[:, :, :, :], AT.Exp, scale=SCALE
            )

            # S = sum_n exp (fp32 tree over n), R = 1/S
            s_tmp = tmppool.tile([Q, H, N // 2, D], F32, tag="tmp")
            nc.vector.tensor_add(
                s_tmp[:, :, :, :],
                e_sb[:, :, 0 : N // 2, :],
                e_sb[:, :, N // 2 : N, :],
            )
            w = N // 2
            while w > 1:
                nc.vector.tensor_add(
                    s_tmp[:, :, 0 : w // 2, :],
                    s_tmp[:, :, 0 : w // 2, :],
                    s_tmp[:, :, w // 2 : w, :],
                )
                w //= 2
            r_sb = smallpool.tile([Q, H, D], F32, tag="r")
            nc.vector.reciprocal(
                r_sb[:, :, :],
                _mk_ap(s_tmp[:], 0, [((N // 2) * D * H, Q), ((N // 2) * D, H), (1, D)]),
            )
            rb_sb = smallpool.tile([Q, H, D], BF16, tag="rb")
            with nc.allow_low_precision("attn normalizer in bf16"):
                nc.vector.tensor_copy(rb_sb[:, :, :], r_sb[:, :, :])

            # attn = exp * (1/S)
            attn_sb = attnpool.tile([Q, H, N, D], BF16, tag="attn")
            nc.vector.tensor_mul(
                attn_sb[:, :, :, :],
                e_sb[:, :, :, :],
                _mk_ap(rb_sb[:], 0, [(H * D, Q), (D, H), (0, N), (1, D)]),
            )
            nc.sync.dma_start(
                out=attn_o[b].rearrange("h q (n d) -> q h n d", n=N),
                in_=attn_sb[:, :, :, :],
            )

            # ctx = (sum_n exp * v) * (1/S) + q
            p_sb = ppool.tile([Q, H, N, D], BF16, tag="p")
            nc.vector.tensor_mul(
                p_sb[:, :, :, :], e_sb[:, :, :, :], v_sb[:, :, :, :]
            )
            c_tmp = tmppool.tile([Q, H, N // 2, D], F32, tag="tmp")
            nc.vector.tensor_add(
                c_tmp[:, :, :, :],
                p_sb[:, :, 0 : N // 2, :],
                p_sb[:, :, N // 2 : N, :],
            )
            w = N // 2
            while w > 1:
                nc.vector.tensor_add(
                    c_tmp[:, :, 0 : w // 2, :],
                    c_tmp[:, :, 0 : w // 2, :],
                    c_tmp[:, :, w // 2 : w, :],
                )
                w //= 2
            ctx_sb = smallpool.tile([Q, F], F32, tag="ctx")
            nc.vector.tensor_mul(
                ctx_sb[:, :],
                _mk_ap(c_tmp[:], 0, [((N // 2) * D * H, Q), ((N // 2) * D, H), (1, D)]),
                r_sb[:, :, :],
            )
            nc.vector.tensor_add(ctx_sb[:, :], ctx_sb[:, :], qf_sb[:, :])
            nc.sync.dma_start(out=ctx_o[b], in_=ctx_sb[:, :])

    nc.compile()
    return nc


def _get_program(with_bd, with_bkbv):
    key = (with_bd, with_bkbv)
    if key not in _cache:
        _cache[key] = _build(with_bd, with_bkbv)
    return _cache[key]


def kernel(atom_query, atom_neighbor, local_distance, mask,
           Wd, bd, Wq, bq, Wk, bk, Wv, bv):
    atom_query = np.asarray(atom_query, np.float32)
    atom_neighbor = np.asarray(atom_neighbor, np.float32)
    local_distance = np.asarray(local_distance, np.float32)
    mask = np.asarray(mask, np.float32)

    bf = ml_dtypes.bfloat16
    an_t = np.ascontiguousarray(
        atom_neighbor.transpose(0, 3, 2, 1)  # [B, f, n, q]
    ).reshape(BS, 2, 128, N, Q).astype(bf)
    ld_p = np.ascontiguousarray(
        local_distance.transpose(0, 1, 3, 2)  # [B, q, dd, n]
    ).astype(bf)
    msk_bf = mask.astype(bf)
    wkwv = np.concatenate(
        [np.asarray(Wk, np.float32), np.asarray(Wv, np.float32)], axis=1
    ).astype(bf)
    wd_bf = np.asarray(Wd, np.float32).astype(bf)
    wq_f = np.asarray(Wq, np.float32)
    bd_f = np.asarray(bd, np.float32).reshape(F)
    bq_f = np.asarray(bq, np.float32).reshape(F)
    bkbv_f = np.concatenate(
        [np.asarray(bk, np.float32).reshape(F), np.asarray(bv, np.float32).reshape(F)]
    )

    with_bd = bool(np.any(bd_f))
    with_bkbv = bool(np.any(bkbv_f))
    nc = _get_program(with_bd, with_bkbv)

    in_maps = []
    for i in range(NC):
        s = slice(i * NB, (i + 1) * NB)
        in_maps.append({
            "an_t": an_t[s], "ld_p": ld_p[s], "msk": msk_bf[s],
            "aq": atom_query[s],
            "wd": wd_bf, "wkwv": wkwv, "wq": wq_f,
            "bq_d": bq_f,
            "bd_bf_d": bd_f.astype(bf),
            "bkbv_d": bkbv_f.astype(bf),
        })
    res = run_bass_kernel_spmd(nc, in_maps, list(range(NC)))

    attn = np.concatenate([r["attn_o"] for r in res.results], axis=0)
    attn = attn.astype(np.float32).reshape(BS, H, Q, N, D)
    ctx = np.concatenate(
        [r["ctx_o"] for r in res.results], axis=0
    ).astype(np.float32)
    return attn, ctx
